# revision 18
# baseline (speedup 1.0000x reference)
"""Trainium2 Bass kernel for a 2-layer TransformerConv GNN (DGT).

Contract: kernel(**inputs) takes the FULL inputs from setup_inputs() and
returns the FULL [N, 2] output.  Internally shards destination nodes (and
their incoming edges) across 8 NeuronCores.

Key algebraic simplification: e_attr = t*time_W[0] + time_b is rank-1 in t,
so the per-layer edge projection e = e_attr @ We + be = t*u + c with
u = time_W[0] @ We, c = time_b @ We + be.  Then
  alpha[e,h] = (q_s[dst]*k[src]).sum_c + t * A[dst,h] + B[dst,h]
with q_s = q/sqrt(C), A = q_s . u (per head), B = q_s . c, and the message
sum reduces to
  out[n,h,:] = (sum_e w*v'[src] + (sum_e w*t) * u_h) / (sum_e w)
with v' = v + c and w = exp(alpha) (softmax max-shift is unnecessary: the
logits are O(1) for this model so exp cannot overflow in fp32).
"""

import math
import sys

sys.path.insert(0, "/opt/trn_rl_repo")

from contextlib import ExitStack

import numpy as np

import concourse.bass as bass
import concourse.tile as tile
from concourse import bacc, mybir
from concourse import library_config
from concourse.bass_utils import run_bass_kernel_spmd
from concourse._compat import cdiv

F32 = mybir.dt.float32
F32R = mybir.dt.float32r
BF16 = mybir.dt.bfloat16
I16 = mybir.dt.int16
AF = mybir.ActivationFunctionType
OP = mybir.AluOpType

HID = 256
HEADS = 8
C = HID // HEADS
LAYERS = 2
P = 128  # partitions


# ---------------------------------------------------------------------------
# Host-side preprocessing
# ---------------------------------------------------------------------------

def _assign_nodes(dst, n_nodes, n_bins, cap):
    """Balanced assignment of nodes to (rank, block) bins.

    Greedy LPT: nodes in decreasing in-degree order go to the open bin
    (nodes < 128) with the fewest edges (soft cap on edges per bin).
    Returns gid_of_node [N] (permuted id) and counts.
    """
    import heapq

    deg = np.bincount(dst, minlength=n_nodes)
    order = np.argsort(-deg, kind="stable")
    heap = [(0, b) for b in range(n_bins)]
    heapq.heapify(heap)
    bin_nodes = np.zeros(n_bins, np.int64)
    bin_edges = np.zeros(n_bins, np.int64)
    gid_of_node = np.empty(n_nodes, np.int64)
    spill = []
    for n in order:
        d = int(deg[n])
        while True:
            if not heap:
                # all bins at edge cap; use least-loaded non-full bin
                heapq.heapify(spill)
                heap.extend(spill)
                spill.clear()
                heapq.heapify(heap)
            e, b = heapq.heappop(heap)
            if bin_nodes[b] >= P:
                continue
            if e + d > cap and bin_nodes[b] < P:
                spill.append((e, b))
                if heap:
                    continue
                else:
                    # no bin under cap; put it in the least loaded anyway
                    heapq.heapify(spill)
                    e, b = heapq.heappop(spill)
            break
        gid_of_node[n] = b * P + bin_nodes[b]
        bin_nodes[b] += 1
        bin_edges[b] += d
        if bin_nodes[b] < P:
            heapq.heappush(heap, (bin_edges[b], b))
    assert (bin_nodes <= P).all()
    return gid_of_node


def preprocess(x, edge_index, edge_time, params, R=8):
    """All host-side index/layout preparation. Returns (meta, per-core input
    maps building blocks)."""
    N, IN = x.shape
    E = edge_index.shape[1]
    assert IN == HID
    NB = cdiv(N, R * P)           # node blocks per rank
    NPR = NB * P                  # nodes per rank (padded)
    NPAD = R * NPR
    W0END = min(NPAD, 32768)
    W1BASE = max(0, NPAD - 32768)
    cap = 2 * cdiv(E, 2 * R * NB)  # soft edges-per-block cap

    src = np.asarray(edge_index[0], np.int64)
    dst = np.asarray(edge_index[1], np.int64)
    t = np.asarray(edge_time, np.float32)

    gid_of_node = _assign_nodes(dst, N, R * NB, cap)
    # append dummy ids for padded gids (no real node)
    node_of_gid = np.full(NPAD, -1, np.int64)
    node_of_gid[gid_of_node] = np.arange(N)

    src_pid = gid_of_node[src]
    dst_pid = gid_of_node[dst]
    e_rank = dst_pid // NPR
    e_block = (dst_pid % NPR) // P
    e_slot = dst_pid % P

    # --- window assignment (for int16 gather indices) ---
    # fixed0: src_pid < W1BASE must use window 0; >= W0END must use window 1.
    # per (rank, block): balance the two windows.
    ebw_lists = [[[None, None] for _ in range(NB)] for _ in range(R)]
    for r in range(R):
        for b in range(NB):
            sel = np.where((e_rank == r) & (e_block == b))[0]
            sp = src_pid[sel]
            fixed0 = sel[sp < W1BASE]
            fixed1 = sel[sp >= W0END]
            flex = sel[(sp >= W1BASE) & (sp < W0END)]
            T = sel.size
            n0 = min(max((T + 1) // 2, fixed0.size), T - fixed1.size)
            take0 = n0 - fixed0.size
            w0 = np.concatenate([fixed0, flex[:take0]])
            w1 = np.concatenate([fixed1, flex[take0:]])
            ebw_lists[r][b][0] = w0
            ebw_lists[r][b][1] = w1

    # group sizes: max over ranks so the SPMD program is shared
    n_bw = np.zeros((NB, 2), np.int64)
    for b in range(NB):
        for w in range(2):
            n_bw[b, w] = max(max(ebw_lists[r][b][w].size for r in range(R)), 1)

    ntiles_bw = np.maximum(1, (n_bw + P - 1) // P)
    NTT = int(ntiles_bw.sum())          # total edge tiles per layer
    TOTC = int(((n_bw + 15) // 16).sum())  # total idx columns

    # --- per-rank packed arrays ---
    kvidx = np.zeros((R, 16, TOTC), np.int16)
    slots = np.full((R, P, NTT), 255.0, np.float32)
    tvals = np.zeros((R, P, NTT), np.float32)

    col_off = np.zeros((NB, 2), np.int64)
    tile_off = np.zeros((NB, 2), np.int64)
    co = 0
    to = 0
    for b in range(NB):
        for w in range(2):
            col_off[b, w] = co
            tile_off[b, w] = to
            co += int((n_bw[b, w] + 15) // 16)
            to += int(ntiles_bw[b, w])

    wbase = [0, W1BASE]
    for r in range(R):
        for b in range(NB):
            for w in range(2):
                el = ebw_lists[r][b][w]
                n = int(n_bw[b, w])
                k = el.size
                idxs = np.zeros(n, np.int64)
                idxs[:k] = src_pid[el] - wbase[w]
                sl = np.full(n, 255.0, np.float32)
                sl[:k] = e_slot[el].astype(np.float32)
                tv = np.zeros(n, np.float32)
                tv[:k] = t[el]
                assert idxs.min() >= 0 and idxs.max() < 32768
                # idx i lives at [i % 16, col_off + i // 16]
                ncol = (n + 15) // 16
                buf = np.zeros(16 * ncol, np.int64)
                buf[:n] = idxs
                kvidx[r, :, col_off[b, w]:col_off[b, w] + ncol] = (
                    buf.reshape(ncol, 16).T.astype(np.int16))
                # slot/t for tile tau at column tile_off+tau, rows 0:K
                nt = int(ntiles_bw[b, w])
                sbuf = np.full(nt * P, 255.0, np.float32)
                sbuf[:n] = sl
                tbuf = np.zeros(nt * P, np.float32)
                tbuf[:n] = tv
                slots[r, :, tile_off[b, w]:tile_off[b, w] + nt] = (
                    sbuf.reshape(nt, P).T)
                tvals[r, :, tile_off[b, w]:tile_off[b, w] + nt] = (
                    tbuf.reshape(nt, P).T)

    kvidx = np.tile(kvidx, (1, 8, 1))  # replicate to 128 partitions

    # --- node features (permuted + padded), blocked transpose ---
    xpad = np.zeros((NPAD, HID), np.float32)
    xpad[gid_of_node] = np.asarray(x, np.float32)
    xT_blk = np.zeros((R * HID, NPR), np.float32)
    for r in range(R):
        xT_blk[r * HID:(r + 1) * HID] = xpad[r * NPR:(r + 1) * NPR].T

    # --- weights ---
    time_w = np.asarray(params['time_W'], np.float32)[:, :]  # [1, HID]
    time_b = np.asarray(params['time_b'], np.float32)
    lw = []
    s = 1.0 / math.sqrt(C)
    for l, pl in enumerate(params['layers']):
        Wq = np.asarray(pl['Wq'], np.float32)
        bq = np.asarray(pl['bq'], np.float32)
        Wk = np.asarray(pl['Wk'], np.float32)
        bk = np.asarray(pl['bk'], np.float32)
        Wv = np.asarray(pl['Wv'], np.float32)
        bv = np.asarray(pl['bv'], np.float32)
        We = np.asarray(pl['We'], np.float32)
        be = np.asarray(pl['be'], np.float32)
        Ws = np.asarray(pl['Ws'], np.float32)
        bs = np.asarray(pl['bs'], np.float32)
        u = time_w[0] @ We                       # [HID]
        c = time_b @ We + be                     # [HID]
        Wq_s, bq_s = Wq * s, bq * s
        Umat = np.zeros((HID, HEADS), np.float32)
        Cmat = np.zeros((HID, HEADS), np.float32)
        for h in range(HEADS):
            Umat[h * C:(h + 1) * C, h] = u[h * C:(h + 1) * C]
            Cmat[h * C:(h + 1) * C, h] = c[h * C:(h + 1) * C]
        W_qe = np.concatenate([Wq_s, Wq_s @ Umat, Wq_s @ Cmat], 1)  # [HID,272]
        b_qe = np.concatenate([bq_s, bq_s @ Umat, bq_s @ Cmat])
        W_kv = np.concatenate([Wk, Wv], 1)                          # [HID,512]
        b_kv = np.concatenate([bk, bv + c])
        lw.append(dict(W_kv=W_kv, b_kv=b_kv, W_qe=W_qe, b_qe=b_qe,
                       W_sk=Ws, b_sk=bs, u=u))
    out_W = np.asarray(params['out_W'], np.float32)
    out_b = np.asarray(params['out_b'], np.float32)

    meta = dict(R=R, NB=NB, NPR=NPR, NPAD=NPAD, W1BASE=W1BASE,
                n_bw=n_bw, ntiles_bw=ntiles_bw, NTT=NTT, TOTC=TOTC,
                col_off=col_off, tile_off=tile_off)

    import ml_dtypes
    bf16 = ml_dtypes.bfloat16
    iota = np.tile(np.arange(P, dtype=np.float32), (P, 1))
    ident = np.eye(P, dtype=np.float32)
    ident_bf = np.eye(P, dtype=bf16)
    ones_row = np.ones((1, P), np.float32)
    ones_bf = np.ones((1, P), bf16)

    common = dict(xT_blk=xT_blk.astype(bf16), iota=iota, ident=ident,
                  ident_bf=ident_bf, ones_row=ones_row, ones_bf=ones_bf,
                  out_W=out_W.reshape(2, P, 2).transpose(1, 0, 2).reshape(P, 4),
                  out_b=out_b.reshape(1, 2))
    for l in range(LAYERS):
        d = lw[l]
        common[f'W_kv{l}'] = d['W_kv'].reshape(2, P, 512).transpose(1, 0, 2).reshape(P, 1024).astype(bf16)
        common[f'b_kv{l}'] = d['b_kv'].reshape(1, 512).astype(bf16)
        common[f'W_qe{l}'] = d['W_qe'].reshape(2, P, 272).transpose(1, 0, 2).reshape(P, 544).astype(bf16)
        common[f'b_qe{l}'] = d['b_qe'].reshape(1, 272).astype(bf16)
        common[f'W_sk{l}'] = d['W_sk'].reshape(2, P, 256).transpose(1, 0, 2).reshape(P, 512).astype(bf16)
        common[f'b_sk{l}'] = d['b_sk'].reshape(1, 256).astype(bf16)
        common[f'u{l}'] = np.tile(d['u'], (P, 1))

    per_core = []
    for r in range(R):
        m = dict(common)
        m['x_ownT'] = np.ascontiguousarray(xT_blk[r * HID:(r + 1) * HID]).astype(bf16)
        m['kvidx'] = np.ascontiguousarray(kvidx[r])
        m['slots'] = np.ascontiguousarray(slots[r])
        m['tvals'] = np.ascontiguousarray(tvals[r])
        per_core.append(m)

    return meta, per_core, node_of_gid


# ---------------------------------------------------------------------------
# Bass program
# ---------------------------------------------------------------------------

def build_program(meta):
    import os
    no_cc = bool(int(os.environ.get("KERNEL_NOCC", "0")))
    sp = bool(int(os.environ.get("KERNEL_SP", "1")))
    R, NB, NPR, NPAD = meta['R'], meta['NB'], meta['NPR'], meta['NPAD']
    W1BASE = meta['W1BASE']
    n_bw, ntiles_bw = meta['n_bw'], meta['ntiles_bw']
    NTT, TOTC = meta['NTT'], meta['TOTC']
    col_off, tile_off = meta['col_off'], meta['tile_off']

    nc = bacc.Bacc("TRN2", target_bir_lowering=False, debug=False,
                   num_devices=R, enable_asserts=False)

    # ---- DRAM I/O ----
    din = {}
    def dinp(name, shape, dt=F32):
        din[name] = nc.dram_tensor(name, list(shape), dt, kind="ExternalInput")
        return din[name]

    xT_blk = dinp('xT_blk', (R * HID, NPR), BF16)
    x_ownT = dinp('x_ownT', (HID, NPR), BF16)
    kvidx = dinp('kvidx', (P, TOTC), I16)
    slots_d = dinp('slots', (P, NTT))
    tvals_d = dinp('tvals', (P, NTT))
    iota_d = dinp('iota', (P, P))
    ident_d = dinp('ident', (P, P))
    identbf_d = dinp('ident_bf', (P, P), BF16)
    ones_d = dinp('ones_row', (1, P))
    onesbf_d = dinp('ones_bf', (1, P), BF16)
    outw_d = dinp('out_W', (P, 4))
    outb_d = dinp('out_b', (1, 2))
    for l in range(LAYERS):
        dinp(f'W_kv{l}', (P, 1024), BF16); dinp(f'b_kv{l}', (1, 512), BF16)
        dinp(f'W_qe{l}', (P, 544), BF16); dinp(f'b_qe{l}', (1, 272), BF16)
        dinp(f'W_sk{l}', (P, 512), BF16); dinp(f'b_sk{l}', (1, 256), BF16)
        dinp(f'u{l}', (P, HID))

    out_y = nc.dram_tensor('out_y', [NPR, 2], F32, kind="ExternalOutput")

    W0E = min(NPAD, 32768)
    kv_w0 = nc.dram_tensor('kv_w0', [W0E, 512], BF16)
    kv_w1 = nc.dram_tensor('kv_w1', [NPAD - W1BASE, 512], BF16)
    h_ownT = nc.dram_tensor('h_ownT', [HID, NPR], BF16)
    hT_full = nc.dram_tensor('hT_full', [R * HID, NPR], BF16,
                             addr_space="Shared" if R > 4 else "Local")
    h2_ownT = nc.dram_tensor('h2_ownT', [HID, NPR], F32)

    with tile.TileContext(nc) as tc, ExitStack() as ctx:
        cpool = ctx.enter_context(tc.tile_pool(name="const", bufs=1))
        respool = ctx.enter_context(tc.tile_pool(name="res", bufs=1))
        lhpool = ctx.enter_context(tc.tile_pool(name="lh", bufs=6))
        sbig = ctx.enter_context(tc.tile_pool(name="sbig", bufs=3))
        gpool = ctx.enter_context(tc.tile_pool(name="gpool", bufs=3))
        wpool = ctx.enter_context(tc.tile_pool(name="wpool", bufs=4))
        epool = ctx.enter_context(tc.tile_pool(name="epool", bufs=4))
        spool = ctx.enter_context(tc.tile_pool(name="spool", bufs=4))
        ppool = ctx.enter_context(tc.tile_pool(name="post", bufs=3))
        ps_big = ctx.enter_context(tc.tile_pool(name="ps_big", bufs=2, space="PSUM"))
        ps_acc = ctx.enter_context(tc.tile_pool(name="ps_acc", bufs=2, space="PSUM"))
        ps_qe = ctx.enter_context(tc.tile_pool(name="ps_qe", bufs=2, space="PSUM"))
        ps_sm = ctx.enter_context(tc.tile_pool(name="ps_sm", bufs=2, space="PSUM"))

        def load_const(dram, shape, dt=None):
            t_ = cpool.tile(list(shape), dt or dram.dtype, tag=dram.name)
            nc.sync.dma_start(t_[:], dram[:])
            return t_

        iota_sb = load_const(iota_d, (P, P))
        ident_sb = load_const(ident_d, (P, P))
        identbf_sb = load_const(identbf_d, (P, P), BF16)
        ones_sb = load_const(ones_d, (1, P))
        onesbf_sb = load_const(onesbf_d, (1, P), BF16)
        kvidx_sb = load_const(kvidx, (P, TOTC), I16)
        slots_sb = load_const(slots_d, (P, NTT))
        tvals_sb = load_const(tvals_d, (P, NTT))
        outw_sb = load_const(outw_d, (P, 4))
        outb_sb = load_const(outb_d, (1, 2))
        Wl = []
        for l in range(LAYERS):
            Wl.append({k: load_const(din[f'{k}{l}'], din[f'{k}{l}'].shape,)
                       for k in ('W_kv', 'b_kv', 'W_qe', 'b_qe', 'W_sk',
                                 'b_sk', 'u')})

        nc.gpsimd.load_library(library_config.mlp)

        qext_sb = respool.tile([P, NB, 272], BF16, tag="qext")
        skip_sb = respool.tile([P, NB, 256], F32, tag="skip")

        for L in range(LAYERS):
            hT_src = xT_blk if L == 0 else hT_full
            ownT_src = x_ownT if L == 0 else h_ownT
            W = Wl[L]

            # ---- own-node projections: q_ext (q_s|A|B) and skip ----
            for j in range(NB):
                lh = lhpool.tile([P, 2, P], BF16, tag="lh")
                nc.sync.dma_start(
                    lh[:],
                    ownT_src[0:2 * P, j * P:(j + 1) * P]
                    .rearrange("(k p) n -> p k n", k=2))
                ps = ps_qe.tile([P, 272], F32, tag="qe")
                for k in range(2):
                    nc.tensor.matmul(ps[:], lh[:, k, :],
                                     W['W_qe'][:, k * 272:(k + 1) * 272],
                                     start=(k == 0), stop=False)
                nc.tensor.matmul(ps[:], onesbf_sb[:], W['b_qe'][:],
                                 start=False, stop=True)
                nc.scalar.activation(qext_sb[:, j, :], ps[:], AF.Copy)

                ps2 = ps_sm.tile([P, 256], F32, tag="sm")
                for k in range(2):
                    nc.tensor.matmul(ps2[:], lh[:, k, :],
                                     W['W_sk'][:, k * 256:(k + 1) * 256],
                                     start=(k == 0), stop=False)
                nc.tensor.matmul(ps2[:], onesbf_sb[:], W['b_sk'][:],
                                 start=False, stop=True)
                nc.scalar.activation(skip_sb[:, j, :], ps2[:], AF.Copy)

            # ---- full kv table: kv = [k | v + c] for all nodes ----
            for g in range(R * NB):
                rr, jj = divmod(g, NB)
                lh = lhpool.tile([P, 2, P], BF16, tag="lh")
                nc.sync.dma_start(
                    lh[:],
                    hT_src[rr * HID:rr * HID + 2 * P, jj * P:(jj + 1) * P]
                    .rearrange("(k p) n -> p k n", k=2))
                ps = ps_big.tile([P, 512], F32, tag="ps_kv")
                for k in range(2):
                    nc.tensor.matmul(ps[:], lh[:, k, :],
                                     W['W_kv'][:, k * 512:(k + 1) * 512],
                                     start=(k == 0), stop=False)
                nc.tensor.matmul(ps[:], onesbf_sb[:], W['b_kv'][:],
                                 start=False, stop=True)
                sb = sbig.tile([P, 512], BF16, tag="kv_sb")
                nc.scalar.activation(sb[:], ps[:], AF.Copy)
                lo, hi = g * P, (g + 1) * P
                if lo < W0E:
                    nc.sync.dma_start(kv_w0[lo:hi, :], sb[:])
                if hi > W1BASE:
                    nc.sync.dma_start(kv_w1[lo - W1BASE:hi - W1BASE, :], sb[:])

            # ---- edge stage, one destination block at a time ----
            wbase = [0, W1BASE]
            for b in range(NB):
                P_ps = ps_acc.tile([P, 272], F32, tag="P_ps")
                total_mm = int(ntiles_bw[b, 0] + ntiles_bw[b, 1])
                mm = 0
                for w in range(2):
                    n = int(n_bw[b, w])
                    nt = int(ntiles_bw[b, w])
                    ncol = (n + 15) // 16
                    kvt = gpool.tile([P, nt, 512], BF16, tag="kvt")
                    kv_win = kv_w0 if w == 0 else kv_w1
                    nc.gpsimd.dma_gather(
                        kvt[:], kv_win[:, :],
                        kvidx_sb[:, int(col_off[b, w]):int(col_off[b, w]) + ncol],
                        n, n, 512, single_packet=sp)
                    for tau in range(nt):
                        K = min(P, n - P * tau)
                        tcol = int(tile_off[b, w]) + tau
                        # one-hot W[e, n] = (slot[e] == n)
                        Wsb = wpool.tile([P, P], BF16, tag="Wsb")
                        nc.vector.tensor_scalar(
                            Wsb[:], iota_sb[:],
                            slots_sb[:, tcol:tcol + 1], None, OP.is_equal)
                        # WT = W.T via PE
                        WT_ps = ps_sm.tile([P, 512], BF16, tag="sm")
                        nc.tensor.transpose(WT_ps[:, 0:K], Wsb[0:K, :],
                                            identbf_sb[0:K, 0:K])
                        WTsb = wpool.tile([P, P], BF16, tag="WTsb")
                        nc.scalar.activation(WTsb[:, 0:K], WT_ps[:, 0:K], AF.Copy)
                        # q_exp = W @ q_ext_block  -> [K, 272] in PSUM
                        qe = ps_qe.tile([P, 272], F32, tag="qe")
                        nc.tensor.matmul(qe[0:K, :], WTsb[:, 0:K],
                                         qext_sb[:, b, :])
                        # qk dot per head
                        qkb = epool.tile([P, 256], F32, tag="qkb")
                        nc.vector.tensor_tensor(qkb[0:K, :], qe[0:K, 0:256],
                                                kvt[0:K, tau, 0:256], OP.mult)
                        qk = spool.tile([P, 8], F32, tag="qk")
                        nc.vector.tensor_reduce(
                            qk[0:K, :],
                            qkb[0:K, :].rearrange("p (h c) -> p h c", h=8),
                            mybir.AxisListType.X, OP.add)
                        # alpha = qk + t*A + B
                        at = spool.tile([P, 8], F32, tag="at")
                        nc.vector.tensor_scalar(
                            at[0:K, :], qe[0:K, 256:264],
                            tvals_sb[0:K, tcol:tcol + 1], None, OP.mult)
                        ab = spool.tile([P, 8], F32, tag="ab")
                        nc.vector.tensor_tensor(ab[0:K, :], at[0:K, :],
                                                qe[0:K, 264:272], OP.add)
                        alpha = spool.tile([P, 8], F32, tag="alpha")
                        nc.vector.tensor_tensor(alpha[0:K, :], ab[0:K, :],
                                                qk[0:K, :], OP.add)
                        # payload = [w*v' | w | w*t]
                        pay = epool.tile([P, 272], BF16, tag="pay")
                        nc.scalar.activation(pay[0:K, 256:264], alpha[0:K, :],
                                             AF.Exp)
                        nc.vector.tensor_scalar(
                            pay[0:K, 264:272], pay[0:K, 256:264],
                            tvals_sb[0:K, tcol:tcol + 1], None, OP.mult)
                        wb = (pay[0:K, 256:264].unsqueeze(2)
                              .broadcast_to((K, 8, 32)))
                        nc.vector.tensor_tensor(
                            pay[0:K, 0:256].rearrange("p (h c) -> p h c", h=8),
                            kvt[0:K, tau, 256:512].rearrange("p (h c) -> p h c", h=8),
                            wb, OP.mult)
                        # scatter-accumulate into block accumulator
                        nc.tensor.matmul(P_ps[:], Wsb[0:K, :], pay[0:K, :],
                                         start=(mm == 0),
                                         stop=(mm == total_mm - 1),
                                         skip_group_check=True)
                        mm += 1

                # ---- post-process block b ----
                sd = ppool.tile([P, 16], F32, tag="sd")
                nc.scalar.activation(sd[:], P_ps[:, 256:272], AF.Copy)
                dn = ppool.tile([P, 8], F32, tag="dn")
                nc.vector.tensor_scalar(dn[:], sd[:, 0:8], 1e-30, None, OP.add)
                rec = ppool.tile([P, 8], F32, tag="rec")
                nc.vector.reciprocal(rec[:], dn[:])
                tmp = ppool.tile([P, 256], F32, tag="tmp")
                s1b = (sd[:, 8:16].unsqueeze(2)
                       .broadcast_to((P, 8, 32)))
                nc.vector.tensor_tensor(
                    tmp[:].rearrange("p (h c) -> p h c", h=8),
                    Wl[L]['u'][:].rearrange("p (h c) -> p h c", h=8),
                    s1b, OP.mult)
                tmp2 = ppool.tile([P, 256], F32, tag="tmp2")
                nc.vector.tensor_tensor(tmp2[:], tmp[:], P_ps[:, 0:256], OP.add)
                recb = (rec[:].unsqueeze(2).broadcast_to((P, 8, 32)))
                outb_ = ppool.tile([P, 256], F32, tag="outb")
                nc.vector.tensor_tensor(
                    outb_[:].rearrange("p (h c) -> p h c", h=8),
                    tmp2[:].rearrange("p (h c) -> p h c", h=8), recb, OP.mult)
                hsb = ppool.tile([P, 256], F32, tag="hsb")
                nc.vector.tensor_tensor(hsb[:], outb_[:], skip_sb[:, b, :],
                                        OP.add)
                hdt = BF16 if L == 0 else F32
                hid_t = identbf_sb if L == 0 else ident_sb
                hrl = ppool.tile([P, 256], hdt, tag="hrl")
                nc.scalar.activation(hrl[:], hsb[:], AF.Relu)
                # transpose h tile -> h_ownT / h2_ownT
                hT = ppool.tile([P, 2, P], hdt, tag="hT")
                for k in range(2):
                    tp = ps_sm.tile([P, 512], hdt, tag="sm")
                    nc.tensor.transpose(tp[:, 0:P], hrl[:, k * P:(k + 1) * P],
                                        hid_t[:])
                    nc.scalar.activation(hT[:, k, :], tp[:, 0:P], AF.Copy)
                hdst = h_ownT if L == 0 else h2_ownT
                nc.sync.dma_start(
                    hdst[0:2 * P, b * P:(b + 1) * P]
                    .rearrange("(k p) n -> p k n", k=2), hT[:])

            if L == 0 and not no_cc:
                nc.gpsimd.collective_compute(
                    "AllGather", OP.bypass,
                    replica_groups=[list(range(R))],
                    ins=[h_ownT.ap().opt()],
                    outs=[hT_full.ap().opt()],
                )

        # ---- classifier on own nodes ----
        for j in range(NB):
            lh = lhpool.tile([P, 2, P], F32, tag="lh")
            nc.sync.dma_start(
                lh[:],
                h2_ownT[0:2 * P, j * P:(j + 1) * P]
                .rearrange("(k p) n -> p k n", k=2))
            ps = ps_sm.tile([P, 256], F32, tag="sm")
            for k in range(2):
                nc.tensor.matmul(ps[:, 0:2], lh[:, k, :], outw_sb[:, 2 * k:2 * k + 2],
                                 start=(k == 0), stop=False)
            nc.tensor.matmul(ps[:, 0:2], ones_sb[:], outb_sb[:],
                             start=False, stop=True)
            ysb = spool.tile([P, 2], F32, tag="ysb")
            nc.scalar.activation(ysb[:], ps[:, 0:2], AF.Copy)
            nc.sync.dma_start(out_y[j * P:(j + 1) * P, :], ysb[:])

    nc.compile()
    return nc


# ---------------------------------------------------------------------------
# Entry point
# ---------------------------------------------------------------------------

_CACHE = {}


_RUNNER = {}


def _runner_exec(nc, in_maps, n_cores, n_timed=3):
    """Cached shard_map runner (mirrors bass2jax.run_bass_via_pjrt) with
    device-resident inputs, returning outputs + per-call execution seconds."""
    import time
    import jax
    import numpy as _np
    from jax.sharding import Mesh, PartitionSpec
    from jax.experimental.shard_map import shard_map
    from concourse import bass2jax, mybir as _mb
    bass2jax.install_neuronx_cc_hook()

    key = id(nc)
    if key not in _RUNNER:
        pid_name = (nc.partition_id_tensor.name
                    if nc.partition_id_tensor else None)
        in_names, out_names, out_avals, zero_outs = [], [], [], []
        for alloc in nc.m.functions[0].allocations:
            if not isinstance(alloc, _mb.MemoryLocationSet):
                continue
            name = alloc.memorylocations[0].name
            if alloc.kind == "ExternalInput":
                if name != pid_name:
                    in_names.append(name)
            elif alloc.kind == "ExternalOutput":
                out_names.append(name)
                shape = tuple(alloc.tensor_shape)
                dtype = _mb.dt.np(alloc.dtype)
                out_avals.append(jax.core.ShapedArray(shape, dtype))
                zero_outs.append(_np.zeros(shape, dtype))
        n_params = len(in_names)
        all_names = in_names + out_names
        if pid_name is not None:
            all_names = all_names + [pid_name]

        def _body(*args):
            operands = list(args)
            if pid_name is not None:
                operands.append(bass2jax.partition_id_tensor())
            outs = bass2jax._bass_exec_p.bind(
                *operands, out_avals=tuple(out_avals), in_names=tuple(all_names),
                out_names=tuple(out_names), lowering_input_output_aliases=(),
                sim_require_finite=True, sim_require_nnan=True, nc=nc)
            return tuple(outs)

        devices = jax.devices()[:n_cores]
        mesh = Mesh(_np.asarray(devices), ("core",))
        n_outs = len(out_names)
        sharded = jax.jit(
            shard_map(_body, mesh=mesh,
                      in_specs=(PartitionSpec("core"),) * (n_params + n_outs),
                      out_specs=(PartitionSpec("core"),) * n_outs,
                      check_rep=False),
            donate_argnums=tuple(range(n_params, n_params + n_outs)),
            keep_unused=True)
        _RUNNER[key] = (sharded, in_names, out_names, out_avals, zero_outs, mesh)

    sharded, in_names, out_names, out_avals, zero_outs, mesh = _RUNNER[key]
    from jax.sharding import NamedSharding, PartitionSpec as PS
    sh = NamedSharding(mesh, PS("core"))
    n_cores_ = n_cores
    concat_in = [
        jax.device_put(_np.concatenate([_np.asarray(in_maps[c][nm])
                                        for c in range(n_cores_)], axis=0), sh)
        for nm in in_names]
    import jax as _jax
    for a in concat_in:
        a.block_until_ready()

    def mk_zeros():
        return [jax.device_put(
            _np.zeros((n_cores_ * z.shape[0], *z.shape[1:]), z.dtype), sh)
            for z in zero_outs]

    # warm-up call
    z = mk_zeros()
    outs = sharded(*concat_in, *z)
    for o in outs:
        o.block_until_ready()
    result = [
        {nm: _np.asarray(outs[i]).reshape(n_cores_, *out_avals[i].shape)[c]
         for i, nm in enumerate(out_names)} for c in range(n_cores_)]

    secs = []
    for _ in range(n_timed):
        z = mk_zeros()
        for zz in z:
            zz.block_until_ready()
        t0 = time.perf_counter()
        outs = sharded(*concat_in, *z)
        for o in outs:
            o.block_until_ready()
        secs.append(time.perf_counter() - t0)
    return result, secs



def _run(x, edge_index, edge_time, params, R=8, use_sim=False):
    meta, per_core, node_of_gid = preprocess(x, edge_index, edge_time, params, R)
    import os
    key = (R, meta['NPAD'], meta['NTT'], meta['TOTC'],
           os.environ.get("KERNEL_NOCC", "0"), os.environ.get("KERNEL_SP", "1"),
           tuple(meta['n_bw'].ravel().tolist()))
    if key not in _CACHE:
        _CACHE[key] = build_program(meta)
    nc = _CACHE[key]

    if use_sim == "runner":
        outs, secs = _runner_exec(nc, per_core, R)
        outs = [o['out_y'] for o in outs]
        res = ("timed", secs)
    elif use_sim:
        from concourse.bass_interp import MultiCoreSim
        sim = MultiCoreSim(nc, num_cores=R)
        for r in range(R):
            for k, v in per_core[r].items():
                sim.cores[r].tensor(k)[:] = v
        sim.simulate(check_with_hw=False)
        outs = [np.array(sim.cores[r].mem_tensor('out_y')) for r in range(R)]
        res = None
    else:
        import os
        trace = bool(int(os.environ.get("KERNEL_TRACE", "0")))
        res = run_bass_kernel_spmd(nc, per_core, core_ids=list(range(R)),
                                   trace=trace)
        outs = [res.results[r]['out_y'] for r in range(R)]

    NPR = meta['NPR']
    y_pad = np.concatenate(outs, 0)          # [NPAD, 2] in gid order
    N = x.shape[0]
    y = np.empty((N, 2), np.float32)
    valid = node_of_gid >= 0
    y[node_of_gid[valid]] = y_pad[valid]
    return y, res


def kernel(x, edge_index, edge_time, params):
    y, _ = _run(np.asarray(x), np.asarray(edge_index), np.asarray(edge_time),
                params, R=8, use_sim=False)
    return y


# revision 20
# speedup vs baseline: 1.0120x; 1.0120x over previous
"""Trainium2 Bass kernel for a 2-layer TransformerConv GNN (DGT).

Contract: kernel(**inputs) takes the FULL inputs from setup_inputs() and
returns the FULL [N, 2] output.  Internally shards destination nodes (and
their incoming edges) across 8 NeuronCores.

Key algebraic simplification: e_attr = t*time_W[0] + time_b is rank-1 in t,
so the per-layer edge projection e = e_attr @ We + be = t*u + c with
u = time_W[0] @ We, c = time_b @ We + be.  Then
  alpha[e,h] = (q_s[dst]*k[src]).sum_c + t * A[dst,h] + B[dst,h]
with q_s = q/sqrt(C), A = q_s . u (per head), B = q_s . c, and the message
sum reduces to
  out[n,h,:] = (sum_e w*v'[src] + (sum_e w*t) * u_h) / (sum_e w)
with v' = v + c and w = exp(alpha) (softmax max-shift is unnecessary: the
logits are O(1) for this model so exp cannot overflow in fp32).
"""

import math
import sys

sys.path.insert(0, "/opt/trn_rl_repo")

from contextlib import ExitStack

import numpy as np

import concourse.bass as bass
import concourse.tile as tile
from concourse import bacc, mybir
from concourse import library_config
from concourse.bass_utils import run_bass_kernel_spmd
from concourse._compat import cdiv

F32 = mybir.dt.float32
F32R = mybir.dt.float32r
BF16 = mybir.dt.bfloat16
I16 = mybir.dt.int16
AF = mybir.ActivationFunctionType
OP = mybir.AluOpType

HID = 256
HEADS = 8
C = HID // HEADS
LAYERS = 2
P = 128  # partitions


# ---------------------------------------------------------------------------
# Host-side preprocessing
# ---------------------------------------------------------------------------

def _assign_nodes(dst, n_nodes, n_bins, cap):
    """Balanced assignment of nodes to (rank, block) bins.

    Greedy LPT: nodes in decreasing in-degree order go to the open bin
    (nodes < 128) with the fewest edges (soft cap on edges per bin).
    Returns gid_of_node [N] (permuted id) and counts.
    """
    import heapq

    deg = np.bincount(dst, minlength=n_nodes)
    order = np.argsort(-deg, kind="stable")
    heap = [(0, b) for b in range(n_bins)]
    heapq.heapify(heap)
    bin_nodes = np.zeros(n_bins, np.int64)
    bin_edges = np.zeros(n_bins, np.int64)
    gid_of_node = np.empty(n_nodes, np.int64)
    spill = []
    for n in order:
        d = int(deg[n])
        while True:
            if not heap:
                # all bins at edge cap; use least-loaded non-full bin
                heapq.heapify(spill)
                heap.extend(spill)
                spill.clear()
                heapq.heapify(heap)
            e, b = heapq.heappop(heap)
            if bin_nodes[b] >= P:
                continue
            if e + d > cap and bin_nodes[b] < P:
                spill.append((e, b))
                if heap:
                    continue
                else:
                    # no bin under cap; put it in the least loaded anyway
                    heapq.heapify(spill)
                    e, b = heapq.heappop(spill)
            break
        gid_of_node[n] = b * P + bin_nodes[b]
        bin_nodes[b] += 1
        bin_edges[b] += d
        if bin_nodes[b] < P:
            heapq.heappush(heap, (bin_edges[b], b))
    assert (bin_nodes <= P).all()
    return gid_of_node


def preprocess(x, edge_index, edge_time, params, R=8):
    """All host-side index/layout preparation. Returns (meta, per-core input
    maps building blocks)."""
    N, IN = x.shape
    E = edge_index.shape[1]
    assert IN == HID
    NB = cdiv(N, R * P)           # node blocks per rank
    NPR = NB * P                  # nodes per rank (padded)
    NPAD = R * NPR
    W0END = min(NPAD, 32768)
    W1BASE = max(0, NPAD - 32768)
    cap = 2 * cdiv(E, 2 * R * NB)  # soft edges-per-block cap

    src = np.asarray(edge_index[0], np.int64)
    dst = np.asarray(edge_index[1], np.int64)
    t = np.asarray(edge_time, np.float32)

    gid_of_node = _assign_nodes(dst, N, R * NB, cap)
    # append dummy ids for padded gids (no real node)
    node_of_gid = np.full(NPAD, -1, np.int64)
    node_of_gid[gid_of_node] = np.arange(N)

    src_pid = gid_of_node[src]
    dst_pid = gid_of_node[dst]
    e_rank = dst_pid // NPR
    e_block = (dst_pid % NPR) // P
    e_slot = dst_pid % P

    # --- window assignment (for int16 gather indices) ---
    # fixed0: src_pid < W1BASE must use window 0; >= W0END must use window 1.
    # per (rank, block): balance the two windows.
    ebw_lists = [[[None, None] for _ in range(NB)] for _ in range(R)]
    for r in range(R):
        for b in range(NB):
            sel = np.where((e_rank == r) & (e_block == b))[0]
            sp = src_pid[sel]
            fixed0 = sel[sp < W1BASE]
            fixed1 = sel[sp >= W0END]
            flex = sel[(sp >= W1BASE) & (sp < W0END)]
            T = sel.size
            n0 = min(max((T + 1) // 2, fixed0.size), T - fixed1.size)
            take0 = n0 - fixed0.size
            w0 = np.concatenate([fixed0, flex[:take0]])
            w1 = np.concatenate([fixed1, flex[take0:]])
            ebw_lists[r][b][0] = w0
            ebw_lists[r][b][1] = w1

    # group sizes: max over ranks so the SPMD program is shared
    n_bw = np.zeros((NB, 2), np.int64)
    for b in range(NB):
        for w in range(2):
            n_bw[b, w] = max(max(ebw_lists[r][b][w].size for r in range(R)), 1)

    ntiles_bw = np.maximum(1, (n_bw + P - 1) // P)
    NTT = int(ntiles_bw.sum())          # total edge tiles per layer
    TOTC = int(((n_bw + 15) // 16).sum())  # total idx columns

    # --- per-rank packed arrays ---
    kvidx = np.zeros((R, 16, TOTC), np.int16)
    slots = np.full((R, P, NTT), 255.0, np.float32)
    tvals = np.zeros((R, P, NTT), np.float32)

    col_off = np.zeros((NB, 2), np.int64)
    tile_off = np.zeros((NB, 2), np.int64)
    co = 0
    to = 0
    for b in range(NB):
        for w in range(2):
            col_off[b, w] = co
            tile_off[b, w] = to
            co += int((n_bw[b, w] + 15) // 16)
            to += int(ntiles_bw[b, w])

    wbase = [0, W1BASE]
    for r in range(R):
        for b in range(NB):
            for w in range(2):
                el = ebw_lists[r][b][w]
                n = int(n_bw[b, w])
                k = el.size
                idxs = np.zeros(n, np.int64)
                idxs[:k] = src_pid[el] - wbase[w]
                sl = np.full(n, 255.0, np.float32)
                sl[:k] = e_slot[el].astype(np.float32)
                tv = np.zeros(n, np.float32)
                tv[:k] = t[el]
                assert idxs.min() >= 0 and idxs.max() < 32768
                # idx i lives at [i % 16, col_off + i // 16]
                ncol = (n + 15) // 16
                buf = np.zeros(16 * ncol, np.int64)
                buf[:n] = idxs
                kvidx[r, :, col_off[b, w]:col_off[b, w] + ncol] = (
                    buf.reshape(ncol, 16).T.astype(np.int16))
                # slot/t for tile tau at column tile_off+tau, rows 0:K
                nt = int(ntiles_bw[b, w])
                sbuf = np.full(nt * P, 255.0, np.float32)
                sbuf[:n] = sl
                tbuf = np.zeros(nt * P, np.float32)
                tbuf[:n] = tv
                slots[r, :, tile_off[b, w]:tile_off[b, w] + nt] = (
                    sbuf.reshape(nt, P).T)
                tvals[r, :, tile_off[b, w]:tile_off[b, w] + nt] = (
                    tbuf.reshape(nt, P).T)

    kvidx = np.tile(kvidx, (1, 8, 1))  # replicate to 128 partitions

    # --- node features (permuted + padded), blocked transpose ---
    xpad = np.zeros((NPAD, HID), np.float32)
    xpad[gid_of_node] = np.asarray(x, np.float32)
    xT_blk = np.zeros((R * HID, NPR), np.float32)
    for r in range(R):
        xT_blk[r * HID:(r + 1) * HID] = xpad[r * NPR:(r + 1) * NPR].T

    # --- weights ---
    time_w = np.asarray(params['time_W'], np.float32)[:, :]  # [1, HID]
    time_b = np.asarray(params['time_b'], np.float32)
    lw = []
    s = 1.0 / math.sqrt(C)
    for l, pl in enumerate(params['layers']):
        Wq = np.asarray(pl['Wq'], np.float32)
        bq = np.asarray(pl['bq'], np.float32)
        Wk = np.asarray(pl['Wk'], np.float32)
        bk = np.asarray(pl['bk'], np.float32)
        Wv = np.asarray(pl['Wv'], np.float32)
        bv = np.asarray(pl['bv'], np.float32)
        We = np.asarray(pl['We'], np.float32)
        be = np.asarray(pl['be'], np.float32)
        Ws = np.asarray(pl['Ws'], np.float32)
        bs = np.asarray(pl['bs'], np.float32)
        u = time_w[0] @ We                       # [HID]
        c = time_b @ We + be                     # [HID]
        Wq_s, bq_s = Wq * s, bq * s
        Umat = np.zeros((HID, HEADS), np.float32)
        Cmat = np.zeros((HID, HEADS), np.float32)
        for h in range(HEADS):
            Umat[h * C:(h + 1) * C, h] = u[h * C:(h + 1) * C]
            Cmat[h * C:(h + 1) * C, h] = c[h * C:(h + 1) * C]
        W_qe = np.concatenate([Wq_s, Wq_s @ Umat, Wq_s @ Cmat], 1)  # [HID,272]
        b_qe = np.concatenate([bq_s, bq_s @ Umat, bq_s @ Cmat])
        W_kv = np.concatenate([Wk, Wv], 1)                          # [HID,512]
        b_kv = np.concatenate([bk, bv + c])
        lw.append(dict(W_kv=W_kv, b_kv=b_kv, W_qe=W_qe, b_qe=b_qe,
                       W_sk=Ws, b_sk=bs, u=u))
    out_W = np.asarray(params['out_W'], np.float32)
    out_b = np.asarray(params['out_b'], np.float32)

    meta = dict(R=R, NB=NB, NPR=NPR, NPAD=NPAD, W1BASE=W1BASE,
                n_bw=n_bw, ntiles_bw=ntiles_bw, NTT=NTT, TOTC=TOTC,
                col_off=col_off, tile_off=tile_off)

    import ml_dtypes
    bf16 = ml_dtypes.bfloat16
    iota = np.tile(np.arange(P, dtype=np.float32), (P, 1))
    ident = np.eye(P, dtype=np.float32)
    ident_bf = np.eye(P, dtype=bf16)
    ones_row = np.ones((1, P), np.float32)
    ones_bf = np.ones((1, P), bf16)

    common = dict(xT_blk=xT_blk.astype(bf16), iota=iota, ident=ident,
                  ident_bf=ident_bf, ones_row=ones_row, ones_bf=ones_bf,
                  out_W=out_W.reshape(2, P, 2).transpose(1, 0, 2).reshape(P, 4),
                  out_b=out_b.reshape(1, 2))
    for l in range(LAYERS):
        d = lw[l]
        common[f'W_kv{l}'] = d['W_kv'].reshape(2, P, 512).transpose(1, 0, 2).reshape(P, 1024).astype(bf16)
        common[f'b_kv{l}'] = d['b_kv'].reshape(1, 512).astype(bf16)
        common[f'W_qe{l}'] = d['W_qe'].reshape(2, P, 272).transpose(1, 0, 2).reshape(P, 544).astype(bf16)
        common[f'b_qe{l}'] = d['b_qe'].reshape(1, 272).astype(bf16)
        common[f'W_sk{l}'] = d['W_sk'].reshape(2, P, 256).transpose(1, 0, 2).reshape(P, 512).astype(bf16)
        common[f'b_sk{l}'] = d['b_sk'].reshape(1, 256).astype(bf16)
        common[f'u{l}'] = np.tile(d['u'], (P, 1))

    per_core = []
    for r in range(R):
        m = dict(common)
        m['x_ownT'] = np.ascontiguousarray(xT_blk[r * HID:(r + 1) * HID]).astype(bf16)
        m['kvidx'] = np.ascontiguousarray(kvidx[r])
        m['slots'] = np.ascontiguousarray(slots[r])
        m['tvals'] = np.ascontiguousarray(tvals[r])
        per_core.append(m)

    return meta, per_core, node_of_gid


# ---------------------------------------------------------------------------
# Bass program
# ---------------------------------------------------------------------------

def build_program(meta):
    import os
    no_cc = bool(int(os.environ.get("KERNEL_NOCC", "0")))
    sp = bool(int(os.environ.get("KERNEL_SP", "1")))
    R, NB, NPR, NPAD = meta['R'], meta['NB'], meta['NPR'], meta['NPAD']
    W1BASE = meta['W1BASE']
    n_bw, ntiles_bw = meta['n_bw'], meta['ntiles_bw']
    NTT, TOTC = meta['NTT'], meta['TOTC']
    col_off, tile_off = meta['col_off'], meta['tile_off']

    nc = bacc.Bacc("TRN2", target_bir_lowering=False, debug=False,
                   num_devices=R, enable_asserts=False)

    # ---- DRAM I/O ----
    din = {}
    def dinp(name, shape, dt=F32):
        din[name] = nc.dram_tensor(name, list(shape), dt, kind="ExternalInput")
        return din[name]

    xT_blk = dinp('xT_blk', (R * HID, NPR), BF16)
    x_ownT = dinp('x_ownT', (HID, NPR), BF16)
    kvidx = dinp('kvidx', (P, TOTC), I16)
    slots_d = dinp('slots', (P, NTT))
    tvals_d = dinp('tvals', (P, NTT))
    iota_d = dinp('iota', (P, P))
    ident_d = dinp('ident', (P, P))
    identbf_d = dinp('ident_bf', (P, P), BF16)
    ones_d = dinp('ones_row', (1, P))
    onesbf_d = dinp('ones_bf', (1, P), BF16)
    outw_d = dinp('out_W', (P, 4))
    outb_d = dinp('out_b', (1, 2))
    for l in range(LAYERS):
        dinp(f'W_kv{l}', (P, 1024), BF16); dinp(f'b_kv{l}', (1, 512), BF16)
        dinp(f'W_qe{l}', (P, 544), BF16); dinp(f'b_qe{l}', (1, 272), BF16)
        dinp(f'W_sk{l}', (P, 512), BF16); dinp(f'b_sk{l}', (1, 256), BF16)
        dinp(f'u{l}', (P, HID))

    out_y = nc.dram_tensor('out_y', [NPR, 2], F32, kind="ExternalOutput")

    W0E = min(NPAD, 32768)
    kv_w0 = nc.dram_tensor('kv_w0', [W0E, 512], BF16)
    kv_w1 = nc.dram_tensor('kv_w1', [NPAD - W1BASE, 512], BF16)
    NCH = min(7, NB)
    CB = cdiv(NB, NCH)          # blocks per chunk

    def chunk_of(j):
        return j // CB, (j % CB) * P

    cw = [(min(NB, (c + 1) * CB) - c * CB) * P for c in range(NCH)]
    h_own_cs = [nc.dram_tensor(f'h_own_{c}', [HID, cw[c]], BF16)
                for c in range(NCH)]
    hT_full_cs = [nc.dram_tensor(f'hT_full_{c}', [R * HID, cw[c]], BF16,
                                 addr_space="Shared" if R > 4 else "Local")
                  for c in range(NCH)]
    h2_ownT = nc.dram_tensor('h2_ownT', [HID, NPR], F32)

    with tile.TileContext(nc) as tc, ExitStack() as ctx:
        cpool = ctx.enter_context(tc.tile_pool(name="const", bufs=1))
        respool = ctx.enter_context(tc.tile_pool(name="res", bufs=1))
        lhpool = ctx.enter_context(tc.tile_pool(name="lh", bufs=6))
        sbig = ctx.enter_context(tc.tile_pool(name="sbig", bufs=3))
        gpool = ctx.enter_context(tc.tile_pool(name="gpool", bufs=3))
        wpool = ctx.enter_context(tc.tile_pool(name="wpool", bufs=4))
        epool = ctx.enter_context(tc.tile_pool(name="epool", bufs=4))
        spool = ctx.enter_context(tc.tile_pool(name="spool", bufs=4))
        ppool = ctx.enter_context(tc.tile_pool(name="post", bufs=3))
        ps_big = ctx.enter_context(tc.tile_pool(name="ps_big", bufs=2, space="PSUM"))
        ps_acc = ctx.enter_context(tc.tile_pool(name="ps_acc", bufs=2, space="PSUM"))
        ps_qe = ctx.enter_context(tc.tile_pool(name="ps_qe", bufs=2, space="PSUM"))
        ps_sm = ctx.enter_context(tc.tile_pool(name="ps_sm", bufs=2, space="PSUM"))

        def load_const(dram, shape, dt=None):
            t_ = cpool.tile(list(shape), dt or dram.dtype, tag=dram.name)
            nc.sync.dma_start(t_[:], dram[:])
            return t_

        iota_sb = load_const(iota_d, (P, P))
        ident_sb = load_const(ident_d, (P, P))
        identbf_sb = load_const(identbf_d, (P, P), BF16)
        ones_sb = load_const(ones_d, (1, P))
        onesbf_sb = load_const(onesbf_d, (1, P), BF16)
        kvidx_sb = load_const(kvidx, (P, TOTC), I16)
        slots_sb = load_const(slots_d, (P, NTT))
        tvals_sb = load_const(tvals_d, (P, NTT))
        outw_sb = load_const(outw_d, (P, 4))
        outb_sb = load_const(outb_d, (1, 2))
        Wl = []
        for l in range(LAYERS):
            Wl.append({k: load_const(din[f'{k}{l}'], din[f'{k}{l}'].shape,)
                       for k in ('W_kv', 'b_kv', 'W_qe', 'b_qe', 'W_sk',
                                 'b_sk', 'u')})

        nc.gpsimd.load_library(library_config.mlp)

        qext_sb = respool.tile([P, NB, 272], BF16, tag="qext")
        skip_sb = respool.tile([P, NB, 256], F32, tag="skip")

        for L in range(LAYERS):
            W = Wl[L]

            def own_src(j):
                if L == 0:
                    return x_ownT, j * P
                c, off = chunk_of(j)
                return h_own_cs[c], off

            def full_src(rr, jj):
                if L == 0:
                    return xT_blk, rr * HID, jj * P
                c, off = chunk_of(jj)
                return hT_full_cs[c], rr * HID, off

            # ---- own-node projections: q_ext (q_s|A|B) and skip ----
            for j in range(NB):
                osrc, ocol = own_src(j)
                lh = lhpool.tile([P, 2, P], BF16, tag="lh")
                nc.sync.dma_start(
                    lh[:],
                    osrc[0:2 * P, ocol:ocol + P]
                    .rearrange("(k p) n -> p k n", k=2))
                ps = ps_qe.tile([P, 272], F32, tag="qe")
                for k in range(2):
                    nc.tensor.matmul(ps[:], lh[:, k, :],
                                     W['W_qe'][:, k * 272:(k + 1) * 272],
                                     start=(k == 0), stop=False)
                nc.tensor.matmul(ps[:], onesbf_sb[:], W['b_qe'][:],
                                 start=False, stop=True)
                nc.scalar.activation(qext_sb[:, j, :], ps[:], AF.Copy)

                ps2 = ps_sm.tile([P, 256], F32, tag="sm")
                for k in range(2):
                    nc.tensor.matmul(ps2[:], lh[:, k, :],
                                     W['W_sk'][:, k * 256:(k + 1) * 256],
                                     start=(k == 0), stop=False)
                nc.tensor.matmul(ps2[:], onesbf_sb[:], W['b_sk'][:],
                                 start=False, stop=True)
                nc.scalar.activation(skip_sb[:, j, :], ps2[:], AF.Copy)

            # ---- full kv table: kv = [k | v + c] for all nodes ----
            for g in range(R * NB):
                rr, jj = divmod(g, NB)
                fsrc, frow, fcol = full_src(rr, jj)
                lh = lhpool.tile([P, 2, P], BF16, tag="lh")
                nc.sync.dma_start(
                    lh[:],
                    fsrc[frow:frow + 2 * P, fcol:fcol + P]
                    .rearrange("(k p) n -> p k n", k=2))
                ps = ps_big.tile([P, 512], F32, tag="ps_kv")
                for k in range(2):
                    nc.tensor.matmul(ps[:], lh[:, k, :],
                                     W['W_kv'][:, k * 512:(k + 1) * 512],
                                     start=(k == 0), stop=False)
                nc.tensor.matmul(ps[:], onesbf_sb[:], W['b_kv'][:],
                                 start=False, stop=True)
                sb = sbig.tile([P, 512], BF16, tag="kv_sb")
                nc.scalar.activation(sb[:], ps[:], AF.Copy)
                lo, hi = g * P, (g + 1) * P
                if lo < W0E:
                    nc.sync.dma_start(kv_w0[lo:hi, :], sb[:])
                if hi > W1BASE:
                    nc.sync.dma_start(kv_w1[lo - W1BASE:hi - W1BASE, :], sb[:])

            # ---- edge stage, one destination block at a time ----
            wbase = [0, W1BASE]
            for b in range(NB):
                P_ps = ps_acc.tile([P, 272], F32, tag="P_ps")
                total_mm = int(ntiles_bw[b, 0] + ntiles_bw[b, 1])
                mm = 0
                for w in range(2):
                    n = int(n_bw[b, w])
                    nt = int(ntiles_bw[b, w])
                    ncol = (n + 15) // 16
                    kvt = gpool.tile([P, nt, 512], BF16, tag="kvt")
                    kv_win = kv_w0 if w == 0 else kv_w1
                    nc.gpsimd.dma_gather(
                        kvt[:], kv_win[:, :],
                        kvidx_sb[:, int(col_off[b, w]):int(col_off[b, w]) + ncol],
                        n, n, 512, single_packet=sp)
                    for tau in range(nt):
                        K = min(P, n - P * tau)
                        tcol = int(tile_off[b, w]) + tau
                        # one-hot W[e, n] = (slot[e] == n)
                        Wsb = wpool.tile([P, P], BF16, tag="Wsb")
                        nc.vector.tensor_scalar(
                            Wsb[:], iota_sb[:],
                            slots_sb[:, tcol:tcol + 1], None, OP.is_equal)
                        # WT = W.T via PE
                        WT_ps = ps_sm.tile([P, 512], BF16, tag="sm")
                        nc.tensor.transpose(WT_ps[:, 0:K], Wsb[0:K, :],
                                            identbf_sb[0:K, 0:K])
                        WTsb = wpool.tile([P, P], BF16, tag="WTsb")
                        nc.scalar.activation(WTsb[:, 0:K], WT_ps[:, 0:K], AF.Copy)
                        # q_exp = W @ q_ext_block  -> [K, 272] in PSUM
                        qe = ps_qe.tile([P, 272], F32, tag="qe")
                        nc.tensor.matmul(qe[0:K, :], WTsb[:, 0:K],
                                         qext_sb[:, b, :])
                        # qk dot per head
                        qkb = epool.tile([P, 256], F32, tag="qkb")
                        nc.vector.tensor_tensor(qkb[0:K, :], qe[0:K, 0:256],
                                                kvt[0:K, tau, 0:256], OP.mult)
                        qk = spool.tile([P, 8], F32, tag="qk")
                        nc.vector.tensor_reduce(
                            qk[0:K, :],
                            qkb[0:K, :].rearrange("p (h c) -> p h c", h=8),
                            mybir.AxisListType.X, OP.add)
                        # alpha = qk + t*A + B
                        at = spool.tile([P, 8], F32, tag="at")
                        nc.vector.tensor_scalar(
                            at[0:K, :], qe[0:K, 256:264],
                            tvals_sb[0:K, tcol:tcol + 1], None, OP.mult)
                        ab = spool.tile([P, 8], F32, tag="ab")
                        nc.vector.tensor_tensor(ab[0:K, :], at[0:K, :],
                                                qe[0:K, 264:272], OP.add)
                        alpha = spool.tile([P, 8], F32, tag="alpha")
                        nc.vector.tensor_tensor(alpha[0:K, :], ab[0:K, :],
                                                qk[0:K, :], OP.add)
                        # payload = [w*v' | w | w*t]
                        pay = epool.tile([P, 272], BF16, tag="pay")
                        nc.scalar.activation(pay[0:K, 256:264], alpha[0:K, :],
                                             AF.Exp)
                        nc.vector.tensor_scalar(
                            pay[0:K, 264:272], pay[0:K, 256:264],
                            tvals_sb[0:K, tcol:tcol + 1], None, OP.mult)
                        wb = (pay[0:K, 256:264].unsqueeze(2)
                              .broadcast_to((K, 8, 32)))
                        nc.vector.tensor_tensor(
                            pay[0:K, 0:256].rearrange("p (h c) -> p h c", h=8),
                            kvt[0:K, tau, 256:512].rearrange("p (h c) -> p h c", h=8),
                            wb, OP.mult)
                        # scatter-accumulate into block accumulator
                        nc.tensor.matmul(P_ps[:], Wsb[0:K, :], pay[0:K, :],
                                         start=(mm == 0),
                                         stop=(mm == total_mm - 1),
                                         skip_group_check=True)
                        mm += 1

                # ---- post-process block b ----
                sd = ppool.tile([P, 16], F32, tag="sd")
                nc.scalar.activation(sd[:], P_ps[:, 256:272], AF.Copy)
                dn = ppool.tile([P, 8], F32, tag="dn")
                nc.vector.tensor_scalar(dn[:], sd[:, 0:8], 1e-30, None, OP.add)
                rec = ppool.tile([P, 8], F32, tag="rec")
                nc.vector.reciprocal(rec[:], dn[:])
                tmp = ppool.tile([P, 256], F32, tag="tmp")
                s1b = (sd[:, 8:16].unsqueeze(2)
                       .broadcast_to((P, 8, 32)))
                nc.vector.tensor_tensor(
                    tmp[:].rearrange("p (h c) -> p h c", h=8),
                    Wl[L]['u'][:].rearrange("p (h c) -> p h c", h=8),
                    s1b, OP.mult)
                tmp2 = ppool.tile([P, 256], F32, tag="tmp2")
                nc.vector.tensor_tensor(tmp2[:], tmp[:], P_ps[:, 0:256], OP.add)
                recb = (rec[:].unsqueeze(2).broadcast_to((P, 8, 32)))
                outb_ = ppool.tile([P, 256], F32, tag="outb")
                nc.vector.tensor_tensor(
                    outb_[:].rearrange("p (h c) -> p h c", h=8),
                    tmp2[:].rearrange("p (h c) -> p h c", h=8), recb, OP.mult)
                hsb = ppool.tile([P, 256], F32, tag="hsb")
                nc.vector.tensor_tensor(hsb[:], outb_[:], skip_sb[:, b, :],
                                        OP.add)
                hdt = BF16 if L == 0 else F32
                hid_t = identbf_sb if L == 0 else ident_sb
                hrl = ppool.tile([P, 256], hdt, tag="hrl")
                nc.scalar.activation(hrl[:], hsb[:], AF.Relu)
                # transpose h tile -> h_ownT / h2_ownT
                hT = ppool.tile([P, 2, P], hdt, tag="hT")
                for k in range(2):
                    tp = ps_sm.tile([P, 512], hdt, tag="sm")
                    nc.tensor.transpose(tp[:, 0:P], hrl[:, k * P:(k + 1) * P],
                                        hid_t[:])
                    nc.scalar.activation(hT[:, k, :], tp[:, 0:P], AF.Copy)
                if L == 0:
                    c, off = chunk_of(b)
                    nc.sync.dma_start(
                        h_own_cs[c][0:2 * P, off:off + P]
                        .rearrange("(k p) n -> p k n", k=2), hT[:])
                    if (b + 1 == NB or (b + 1) % CB == 0) and not no_cc:
                        nc.gpsimd.collective_compute(
                            "AllGather", OP.bypass,
                            replica_groups=[list(range(R))],
                            ins=[h_own_cs[c].ap().opt()],
                            outs=[hT_full_cs[c].ap().opt()])
                else:
                    nc.sync.dma_start(
                        h2_ownT[0:2 * P, b * P:(b + 1) * P]
                        .rearrange("(k p) n -> p k n", k=2), hT[:])


        # ---- classifier on own nodes ----
        for j in range(NB):
            lh = lhpool.tile([P, 2, P], F32, tag="lh")
            nc.sync.dma_start(
                lh[:],
                h2_ownT[0:2 * P, j * P:(j + 1) * P]
                .rearrange("(k p) n -> p k n", k=2))
            ps = ps_sm.tile([P, 256], F32, tag="sm")
            for k in range(2):
                nc.tensor.matmul(ps[:, 0:2], lh[:, k, :], outw_sb[:, 2 * k:2 * k + 2],
                                 start=(k == 0), stop=False)
            nc.tensor.matmul(ps[:, 0:2], ones_sb[:], outb_sb[:],
                             start=False, stop=True)
            ysb = spool.tile([P, 2], F32, tag="ysb")
            nc.scalar.activation(ysb[:], ps[:, 0:2], AF.Copy)
            nc.sync.dma_start(out_y[j * P:(j + 1) * P, :], ysb[:])

    nc.compile()
    return nc


# ---------------------------------------------------------------------------
# Entry point
# ---------------------------------------------------------------------------

_CACHE = {}


_RUNNER = {}


def _runner_exec(nc, in_maps, n_cores, n_timed=3):
    """Cached shard_map runner (mirrors bass2jax.run_bass_via_pjrt) with
    device-resident inputs, returning outputs + per-call execution seconds."""
    import time
    import jax
    import numpy as _np
    from jax.sharding import Mesh, PartitionSpec
    from jax.experimental.shard_map import shard_map
    from concourse import bass2jax, mybir as _mb
    bass2jax.install_neuronx_cc_hook()

    key = id(nc)
    if key not in _RUNNER:
        pid_name = (nc.partition_id_tensor.name
                    if nc.partition_id_tensor else None)
        in_names, out_names, out_avals, zero_outs = [], [], [], []
        for alloc in nc.m.functions[0].allocations:
            if not isinstance(alloc, _mb.MemoryLocationSet):
                continue
            name = alloc.memorylocations[0].name
            if alloc.kind == "ExternalInput":
                if name != pid_name:
                    in_names.append(name)
            elif alloc.kind == "ExternalOutput":
                out_names.append(name)
                shape = tuple(alloc.tensor_shape)
                dtype = _mb.dt.np(alloc.dtype)
                out_avals.append(jax.core.ShapedArray(shape, dtype))
                zero_outs.append(_np.zeros(shape, dtype))
        n_params = len(in_names)
        all_names = in_names + out_names
        if pid_name is not None:
            all_names = all_names + [pid_name]

        def _body(*args):
            operands = list(args)
            if pid_name is not None:
                operands.append(bass2jax.partition_id_tensor())
            outs = bass2jax._bass_exec_p.bind(
                *operands, out_avals=tuple(out_avals), in_names=tuple(all_names),
                out_names=tuple(out_names), lowering_input_output_aliases=(),
                sim_require_finite=True, sim_require_nnan=True, nc=nc)
            return tuple(outs)

        devices = jax.devices()[:n_cores]
        mesh = Mesh(_np.asarray(devices), ("core",))
        n_outs = len(out_names)
        sharded = jax.jit(
            shard_map(_body, mesh=mesh,
                      in_specs=(PartitionSpec("core"),) * (n_params + n_outs),
                      out_specs=(PartitionSpec("core"),) * n_outs,
                      check_rep=False),
            donate_argnums=tuple(range(n_params, n_params + n_outs)),
            keep_unused=True)
        _RUNNER[key] = (sharded, in_names, out_names, out_avals, zero_outs, mesh)

    sharded, in_names, out_names, out_avals, zero_outs, mesh = _RUNNER[key]
    from jax.sharding import NamedSharding, PartitionSpec as PS
    sh = NamedSharding(mesh, PS("core"))
    n_cores_ = n_cores
    concat_in = [
        jax.device_put(_np.concatenate([_np.asarray(in_maps[c][nm])
                                        for c in range(n_cores_)], axis=0), sh)
        for nm in in_names]
    import jax as _jax
    for a in concat_in:
        a.block_until_ready()

    def mk_zeros():
        return [jax.device_put(
            _np.zeros((n_cores_ * z.shape[0], *z.shape[1:]), z.dtype), sh)
            for z in zero_outs]

    # warm-up call
    z = mk_zeros()
    outs = sharded(*concat_in, *z)
    for o in outs:
        o.block_until_ready()
    result = [
        {nm: _np.asarray(outs[i]).reshape(n_cores_, *out_avals[i].shape)[c]
         for i, nm in enumerate(out_names)} for c in range(n_cores_)]

    secs = []
    for _ in range(n_timed):
        z = mk_zeros()
        for zz in z:
            zz.block_until_ready()
        t0 = time.perf_counter()
        outs = sharded(*concat_in, *z)
        for o in outs:
            o.block_until_ready()
        secs.append(time.perf_counter() - t0)
    return result, secs



def _run(x, edge_index, edge_time, params, R=8, use_sim=False):
    meta, per_core, node_of_gid = preprocess(x, edge_index, edge_time, params, R)
    import os
    key = (R, meta['NPAD'], meta['NTT'], meta['TOTC'],
           os.environ.get("KERNEL_NOCC", "0"), os.environ.get("KERNEL_SP", "1"),
           tuple(meta['n_bw'].ravel().tolist()))
    if key not in _CACHE:
        _CACHE[key] = build_program(meta)
    nc = _CACHE[key]

    if use_sim == "runner":
        outs, secs = _runner_exec(nc, per_core, R)
        outs = [o['out_y'] for o in outs]
        res = ("timed", secs)
    elif use_sim:
        from concourse.bass_interp import MultiCoreSim
        sim = MultiCoreSim(nc, num_cores=R)
        for r in range(R):
            for k, v in per_core[r].items():
                sim.cores[r].tensor(k)[:] = v
        sim.simulate(check_with_hw=False)
        outs = [np.array(sim.cores[r].mem_tensor('out_y')) for r in range(R)]
        res = None
    else:
        import os
        trace = bool(int(os.environ.get("KERNEL_TRACE", "0")))
        res = run_bass_kernel_spmd(nc, per_core, core_ids=list(range(R)),
                                   trace=trace)
        outs = [res.results[r]['out_y'] for r in range(R)]

    NPR = meta['NPR']
    y_pad = np.concatenate(outs, 0)          # [NPAD, 2] in gid order
    N = x.shape[0]
    y = np.empty((N, 2), np.float32)
    valid = node_of_gid >= 0
    y[node_of_gid[valid]] = y_pad[valid]
    return y, res


def kernel(x, edge_index, edge_time, params):
    y, _ = _run(np.asarray(x), np.asarray(edge_index), np.asarray(edge_time),
                params, R=8, use_sim=False)
    return y
